# revision 4
# baseline (speedup 1.0000x reference)
"""Trainium2 Bass kernel: sparse AE encoder (L1 fan-in-1 -> relu/BN -> L2 block-diag
4x4 -> relu/BN -> L3 sparse 256-nnz/TF -> BN), SPMD over 8 NeuronCores.

Strategy: shard the gene/hidden axis across cores (BN1/BN2 fully local since every
core holds all 256 batch rows for its features). All three layers run on the
TensorEngine as dense matmuls with host-packed stationaries:
  L1: per-tile [128 gene, 128 hid] scatter matrix with w1 embedded
  L2: per-tile [128, 128] block-diagonal (4x4 gene blocks) from w2
  L3: dense-ified W3 [4096, 1024] shard (bf16), partial z accumulated in PSUM
Partial z^T is ReduceScattered (each core receives its own 128 TF rows summed over
cores), BN3 is computed locally per-partition, and each core outputs its slice.
"""

import numpy as np
import ml_dtypes

import concourse.bacc as bacc
import concourse.bass as bass
import concourse.tile as tile
import concourse.mybir as mybir
from concourse import bass_utils
from concourse.masks import make_identity

# Problem constants (fixed by the problem spec; do not read spec.json here)
N_GENES = 8192
WM = 4
HID = N_GENES * WM          # 32768
N_TF = 1024
B = 256
EPS = 1e-5

NCORES = 8
GSH = N_GENES // NCORES     # 1024 genes / core
HSH = HID // NCORES         # 4096 hidden rows / core
P = 128
NT = HSH // P               # 32 hidden tiles / core
NGT = GSH // P              # 8 gene tiles / core

BF16 = ml_dtypes.bfloat16
F32 = mybir.dt.float32
BF = mybir.dt.bfloat16
AF = mybir.ActivationFunctionType
OP = mybir.AluOpType

TRACE = False
LAST_RESULT = None

_cache = {}


def _build_graph():
    nc = bacc.Bacc("TRN2", target_bir_lowering=False, debug=False, num_devices=NCORES)

    xT = nc.dram_tensor("xT", [GSH, B], BF, kind="ExternalInput").ap()
    e1 = nc.dram_tensor("e1", [NT, P, P], BF, kind="ExternalInput").ap()
    w2b = nc.dram_tensor("w2b", [NT, P, P], BF, kind="ExternalInput").ap()
    w3b = nc.dram_tensor("w3b", [NT, P, N_TF], BF, kind="ExternalInput").ap()
    b1s = nc.dram_tensor("b1s", [P, NT], F32, kind="ExternalInput").ap()
    b2s = nc.dram_tensor("b2s", [P, NT], F32, kind="ExternalInput").ap()
    outT = nc.dram_tensor("outT", [P, B], F32, kind="ExternalOutput").ap()

    from contextlib import ExitStack
    with tile.TileContext(nc) as tc, ExitStack() as ctx:
        cpool = ctx.enter_context(tc.tile_pool(name="const", bufs=1))
        wpool = ctx.enter_context(tc.tile_pool(name="wts", bufs=1))
        apool = ctx.enter_context(tc.tile_pool(name="acts", bufs=1))
        spool = ctx.enter_context(tc.tile_pool(name="stats", bufs=1))
        scrpool = ctx.enter_context(tc.tile_pool(name="scrap", bufs=1))
        ztpool = ctx.enter_context(tc.tile_pool(name="ztile", bufs=3))
        psAB = ctx.enter_context(tc.tile_pool(name="psAB", bufs=2, space="PSUM"))
        psZp = ctx.enter_context(tc.tile_pool(name="psZ", bufs=1, space="PSUM"))
        psTp = ctx.enter_context(tc.tile_pool(name="psT", bufs=2, space="PSUM"))
        dpool = ctx.enter_context(tc.tile_pool(name="dram", bufs=1, space="DRAM"))

        # ---- static loads ------------------------------------------------
        xs = wpool.tile([P, NGT * B], BF, name="xs")
        for g in range(NGT):
            nc.sync.dma_start(xs[:, g * B:(g + 1) * B], xT[g * P:(g + 1) * P, :])
        b1t = cpool.tile([P, NT], F32, name="b1t")
        nc.sync.dma_start(b1t[:], b1s[:])
        b2t = cpool.tile([P, NT], F32, name="b2t")
        nc.sync.dma_start(b2t[:], b2s[:])
        e1s = wpool.tile([P, NT * P], BF, name="e1s")
        w2s = wpool.tile([P, NT * P], BF, name="w2s")
        for t in range(NT):
            nc.sync.dma_start(e1s[:, t * P:(t + 1) * P], e1[t])
        for t in range(NT):
            nc.sync.dma_start(w2s[:, t * P:(t + 1) * P], w2b[t])
        w3s = wpool.tile([P, NT * N_TF], BF, name="w3s")
        for t in range(NT):
            nc.sync.dma_start(w3s[:, t * N_TF:(t + 1) * N_TF], w3b[t])

        idt = cpool.tile([P, P], BF, name="idt")
        make_identity(nc, idt[:])
        epst = cpool.tile([P, 1], F32, name="epst")
        nc.gpsimd.memset(epst[:], EPS)

        hr = apool.tile([P, NT * B], BF, name="hr")
        h1n = apool.tile([P, NT * B], BF, name="h1n")
        h2n = apool.tile([P, NT * B], BF, name="h2n")
        scr = scrpool.tile([P, B], BF, name="scr")

        def phase(lhs_s, rhs_get, btile, dst, hrbuf):
            """One sparse-linear layer + relu + batchnorm, producing bf16 dst."""
            sums = spool.tile([P, NT], F32, name="sums", tag="sums")
            sqs = spool.tile([P, NT], F32, name="sqs", tag="sqs")
            for t in range(NT):
                ps = psAB.tile([P, B], F32, name="psL", tag="psL")
                nc.tensor.matmul(ps[:], lhsT=lhs_s[:, t * P:(t + 1) * P],
                                 rhs=rhs_get(t), start=True, stop=True)
                hrt = hrbuf[:, t * B:(t + 1) * B]
                nc.scalar.activation(hrt, ps[:], AF.Relu,
                                     bias=btile[:, t:t + 1],
                                     accum_out=sums[:, t:t + 1])
                nc.vector.scalar_tensor_tensor(scr[:], in0=hrt, scalar=1.0,
                                               in1=hrt, op0=OP.mult, op1=OP.mult,
                                               accum_out=sqs[:, t:t + 1])
            # batched stats -> istd / (-mean*istd) for all NT tiles at once
            mn = spool.tile([P, NT], F32, name="mn", tag="mn")
            nc.vector.tensor_scalar_mul(mn[:], sums[:], 1.0 / B)
            mq = spool.tile([P, NT], F32, name="mq", tag="mq")
            nc.vector.tensor_scalar_mul(mq[:], sqs[:], 1.0 / B)
            nmsq = spool.tile([P, NT], F32, name="nmsq", tag="nmsq")
            nc.vector.scalar_tensor_tensor(nmsq[:], in0=mn[:], scalar=-1.0,
                                           in1=mn[:], op0=OP.mult, op1=OP.mult)
            var0 = spool.tile([P, NT], F32, name="var0", tag="var0")
            nc.vector.tensor_tensor(var0[:], mq[:], nmsq[:], op=OP.add)
            std = spool.tile([P, NT], F32, name="std", tag="std")
            nc.scalar.activation(std[:], var0[:], AF.Sqrt, bias=epst[:, 0:1])
            istd = spool.tile([P, NT], F32, name="istd", tag="istd")
            nc.vector.reciprocal(istd[:], std[:])
            nm = spool.tile([P, NT], F32, name="nm", tag="nm")
            nc.vector.scalar_tensor_tensor(nm[:], in0=mn[:], scalar=-1.0,
                                           in1=istd[:], op0=OP.mult, op1=OP.mult)
            for t in range(NT):
                nc.vector.tensor_scalar(out=dst[:, t * B:(t + 1) * B],
                                        in0=hrbuf[:, t * B:(t + 1) * B],
                                        scalar1=istd[:, t:t + 1],
                                        scalar2=nm[:, t:t + 1],
                                        op0=OP.mult, op1=OP.add)

        # ---- layer 1 + BN1, layer 2 + BN2 -------------------------------
        phase(e1s, lambda t: xs[:, (t // 4) * B:(t // 4 + 1) * B], b1t, h1n, hr)
        phase(w2s, lambda t: h1n[:, t * B:(t + 1) * B], b2t, h2n, hr)

        # ---- layer 3: z[b, t] partial over this core's hidden shard -----
        psZ = [[psZp.tile([P, 512], F32, name=f"psZ{bh}{th}", tag=f"psZ{bh}{th}")
                for th in range(2)] for bh in range(2)]
        for t in range(NT):
            for bh in range(2):
                lhsT = h2n[:, t * B + bh * P: t * B + (bh + 1) * P]
                for th in range(2):
                    nc.tensor.matmul(psZ[bh][th][:], lhsT=lhsT,
                                     rhs=w3s[:, t * N_TF + th * 512: t * N_TF + (th + 1) * 512],
                                     start=(t == 0), stop=(t == NT - 1))

        # copy PSUM z -> sbuf bf16 [128b, 2*1024] (bh-major)
        zpart = apool.tile([P, 2 * N_TF], BF, name="zpart")
        for bh in range(2):
            for th in range(2):
                nc.vector.tensor_copy(
                    zpart[:, bh * N_TF + th * 512: bh * N_TF + (th + 1) * 512],
                    psZ[bh][th][:])

        # transpose to z^T [1024 tf, 256 b] and bounce to DRAM
        zinT = dpool.tile([N_TF, B], BF, name="zinT")
        for tt in range(N_TF // P):
            zTs = ztpool.tile([P, B], BF, name="zTs", tag="zTs")
            for bh in range(2):
                pst = psTp.tile([P, P], BF, name="pst", tag="pst")
                nc.tensor.transpose(pst[:], in_=zpart[:, bh * N_TF + tt * P: bh * N_TF + (tt + 1) * P],
                                    identity=idt[:])
                nc.vector.tensor_copy(zTs[:, bh * P:(bh + 1) * P], pst[:])
            nc.sync.dma_start(zinT[tt * P:(tt + 1) * P, :], zTs[:])

        # ReduceScatter: core c receives sum over cores of z^T rows [128c, 128c+128)
        zrs = dpool.tile([P, B], BF, name="zrs")
        nc.gpsimd.collective_compute(
            "ReduceScatter", OP.add,
            replica_groups=[list(range(NCORES))],
            ins=[zinT.opt()], outs=[zrs.opt()])

        # ---- BN3 on the local TF slice ----------------------------------
        zsl = ztpool.tile([P, B], BF, name="zsl", tag="zsl")
        nc.sync.dma_start(zsl[:], zrs[:])
        st6 = spool.tile([P, 6], F32, name="st6", tag="st6")
        nc.vector.bn_stats(st6[:], zsl[:])
        mv = spool.tile([P, 2], F32, name="mv", tag="mv")
        nc.vector.bn_aggr(mv[:], st6[:])
        std3 = spool.tile([P, 1], F32, name="std3", tag="std3")
        nc.scalar.activation(std3[:], mv[:, 1:2], AF.Sqrt, bias=epst[:, 0:1])
        istd3 = spool.tile([P, 1], F32, name="istd3", tag="istd3")
        nc.vector.reciprocal(istd3[:], std3[:])
        nm3 = spool.tile([P, 1], F32, name="nm3", tag="nm3")
        nc.vector.scalar_tensor_tensor(nm3[:], in0=mv[:, 0:1], scalar=-1.0,
                                       in1=istd3[:], op0=OP.mult, op1=OP.mult)
        ofin = ztpool.tile([P, B], F32, name="ofin", tag="ofin")
        nc.vector.tensor_scalar(out=ofin[:], in0=zsl[:], scalar1=istd3[:],
                                scalar2=nm3[:], op0=OP.mult, op1=OP.add)
        nc.sync.dma_start(outT[:], ofin[:])

    nc.compile()
    return nc


def _pack_inputs(features, w1, b1, w2, b2, w3, b3,
                 rows1, cols1, rows2, cols2, rows3, cols3):
    """Host-side packing of weights/inputs into per-core padded dense tiles."""
    f32 = np.float32
    features = np.asarray(features, f32)
    w1 = np.asarray(w1, f32); b1 = np.asarray(b1, f32)
    w2 = np.asarray(w2, f32); b2 = np.asarray(b2, f32)
    w3 = np.asarray(w3, f32); b3 = np.asarray(b3, f32)
    rows1 = np.asarray(rows1); cols1 = np.asarray(cols1)
    rows2 = np.asarray(rows2); cols2 = np.asarray(cols2)
    rows3 = np.asarray(rows3); cols3 = np.asarray(cols3)

    # fan-in-1 layer 1: per-row weight and source gene
    w1r = np.empty(HID, f32); w1r[rows1] = w1
    c1r = np.empty(HID, np.int64); c1r[rows1] = cols1

    # layer 2 entries sorted by output row
    order2 = np.argsort(rows2, kind="stable")
    r2 = rows2[order2]; c2 = cols2[order2]; v2 = w2[order2]

    # dense W3 [HID, N_TF]
    W3d = np.zeros((HID, N_TF), f32)
    np.add.at(W3d, (cols3.astype(np.int64), rows3.astype(np.int64)), w3)

    in_maps = []
    for c in range(NCORES):
        hbase = c * HSH
        gbase = c * GSH
        xT = np.ascontiguousarray(features[:, gbase:gbase + GSH].T).astype(BF16)

        e1 = np.zeros((NT, P, P), f32)
        w2t = np.zeros((NT, P, P), f32)
        for t in range(NT):
            R0 = hbase + t * P
            G0 = gbase + (t // 4) * P
            rows = np.arange(R0, R0 + P)
            e1[t][c1r[rows] - G0, np.arange(P)] = w1r[rows]
            es = slice(WM * R0, WM * (R0 + P))
            np.add.at(w2t[t], (c2[es] - R0, r2[es] - R0), v2[es])

        w3t = W3d[hbase:hbase + HSH].reshape(NT, P, N_TF)

        in_maps.append({
            "xT": xT,
            "e1": e1.astype(BF16),
            "w2b": w2t.astype(BF16),
            "w3b": w3t.astype(BF16),
            "b1s": np.ascontiguousarray(b1[hbase:hbase + HSH].reshape(NT, P).T),
            "b2s": np.ascontiguousarray(b2[hbase:hbase + HSH].reshape(NT, P).T),
        })
    return in_maps, b3


def kernel(**inputs) -> np.ndarray:
    global LAST_RESULT
    if "nc" not in _cache:
        _cache["nc"] = _build_graph()
    nc = _cache["nc"]

    in_maps, b3 = _pack_inputs(**inputs)
    # fold b3 into W3? no: b3 is added before BN3; BN3 subtracts the mean, so a
    # per-TF constant bias cancels exactly. It can be (and is) dropped.

    res = bass_utils.run_bass_kernel_spmd(
        nc, in_maps, core_ids=list(range(NCORES)), trace=TRACE)
    LAST_RESULT = res

    outT = np.concatenate([res.results[c]["outT"] for c in range(NCORES)], axis=0)
    return np.ascontiguousarray(outT.T.astype(np.float32))


# revision 6
# speedup vs baseline: 1.2216x; 1.2216x over previous
"""Trainium2 Bass kernel: sparse AE encoder (L1 fan-in-1 -> relu/BN -> L2 block-diag
4x4 -> relu/BN -> L3 sparse 256-nnz/TF -> BN), SPMD over 8 NeuronCores.

Sharding: gene/hidden axis across cores (BN1/BN2 local: every core holds all 256
batch rows of its features). All layers are TensorEngine matmuls with host-packed
stationaries (L1 scatter matrix, L2 block-diagonal, L3 densified W3 shard in bf16).
Partial z is transposed on PE, AllToAll'd, reduced in fp32 on-core, BN3 applied to
the local 128-TF slice, and each core emits its outT shard.
"""

import numpy as np
import ml_dtypes

import concourse.bacc as bacc
import concourse.bass as bass
import concourse.tile as tile
import concourse.mybir as mybir
from concourse import bass_utils
from concourse.masks import make_identity

N_GENES = 8192
WM = 4
HID = N_GENES * WM          # 32768
N_TF = 1024
B = 256
EPS = 1e-5

NCORES = 8
GSH = N_GENES // NCORES     # 1024 genes / core
HSH = HID // NCORES         # 4096 hidden rows / core
P = 128
NT = HSH // P               # 32 hidden tiles / core
NGT = GSH // P              # 8 gene tiles / core
GB = 8                      # stats batching group size (tiles)

BF16 = ml_dtypes.bfloat16
F32 = mybir.dt.float32
BF = mybir.dt.bfloat16
AF = mybir.ActivationFunctionType
OP = mybir.AluOpType

TRACE = False
LAST_RESULT = None

_cache = {}


def _build_graph():
    nc = bacc.Bacc("TRN2", target_bir_lowering=False, debug=False, num_devices=NCORES)

    xd = nc.dram_tensor("xd", [P, NGT * B], BF, kind="ExternalInput").ap()
    e1d = nc.dram_tensor("e1d", [P, NT * P], BF, kind="ExternalInput").ap()
    w2d = nc.dram_tensor("w2d", [P, NT * P], BF, kind="ExternalInput").ap()
    w3d = nc.dram_tensor("w3d", [P, NT * N_TF], BF, kind="ExternalInput").ap()
    b1d = nc.dram_tensor("b1d", [P, NT], F32, kind="ExternalInput").ap()
    b2d = nc.dram_tensor("b2d", [P, NT], F32, kind="ExternalInput").ap()
    outT = nc.dram_tensor("outT", [P, B], F32, kind="ExternalOutput").ap()

    from contextlib import ExitStack
    with tile.TileContext(nc) as tc, ExitStack() as ctx:
        cpool = ctx.enter_context(tc.tile_pool(name="const", bufs=1))
        wpool = ctx.enter_context(tc.tile_pool(name="wts", bufs=1))
        apool = ctx.enter_context(tc.tile_pool(name="acts", bufs=1))
        spool = ctx.enter_context(tc.tile_pool(name="stats", bufs=1))
        ztpool = ctx.enter_context(tc.tile_pool(name="ztile", bufs=3))
        psAB = ctx.enter_context(tc.tile_pool(name="psAB", bufs=2, space="PSUM"))
        psZp = ctx.enter_context(tc.tile_pool(name="psZ", bufs=1, space="PSUM"))
        psTp = ctx.enter_context(tc.tile_pool(name="psT", bufs=2, space="PSUM"))
        dpool = ctx.enter_context(tc.tile_pool(name="dram", bufs=1, space="DRAM"))

        # ---- static loads (contiguous, few instructions) -----------------
        xs = wpool.tile([P, NGT * B], BF, name="xs")
        nc.sync.dma_start(xs[:], xd[:])
        b1t = cpool.tile([P, NT], F32, name="b1t")
        nc.sync.dma_start(b1t[:], b1d[:])
        e1s = wpool.tile([P, NT * P], BF, name="e1s")
        nc.sync.dma_start(e1s[:], e1d[:])
        b2t = cpool.tile([P, NT], F32, name="b2t")
        nc.sync.dma_start(b2t[:], b2d[:])
        w2s = wpool.tile([P, NT * P], BF, name="w2s")
        nc.sync.dma_start(w2s[:], w2d[:])
        w3s = wpool.tile([P, NT * N_TF], BF, name="w3s")
        W3CH = 8
        cw = NT * N_TF // W3CH
        for c in range(W3CH):
            nc.sync.dma_start(w3s[:, c * cw:(c + 1) * cw], w3d[:, c * cw:(c + 1) * cw])

        idt = cpool.tile([P, P], BF, name="idt")
        make_identity(nc, idt[:])
        epst = cpool.tile([P, 1], F32, name="epst")
        nc.gpsimd.memset(epst[:], EPS)

        hr = apool.tile([P, NT * B], BF, name="hr")
        h1n = apool.tile([P, NT * B], BF, name="h1n")
        h2n = apool.tile([P, NT * B], BF, name="h2n")

        def phase(lhs_s, rhs_get, btile, dst, hrbuf):
            """sparse-linear + relu + batchnorm -> bf16 dst, stats batched per GB tiles."""
            st = spool.tile([P, NT * 6], F32, name="st", tag="st")
            mv = spool.tile([P, NT * 2], F32, name="mv", tag="mv")
            for g0 in range(0, NT, GB):
                for t in range(g0, g0 + GB):
                    ps = psAB.tile([P, B], F32, name="psL", tag="psL")
                    nc.tensor.matmul(ps[:], lhsT=lhs_s[:, t * P:(t + 1) * P],
                                     rhs=rhs_get(t), start=True, stop=True)
                    hrt = hrbuf[:, t * B:(t + 1) * B]
                    nc.scalar.activation(hrt, ps[:], AF.Relu, bias=btile[:, t:t + 1])
                    nc.vector.bn_stats(st[:, t * 6:(t + 1) * 6], hrt)
                    nc.vector.bn_aggr(mv[:, t * 2:(t + 1) * 2], st[:, t * 6:(t + 1) * 6])
                # batched istd / (-mean*istd) for the group (strided slices of mv)
                mvg = mv[:, g0 * 2:(g0 + GB) * 2]
                mng = mvg.rearrange("p (t two) -> p two t", two=2)
                std = spool.tile([P, GB], F32, name="std", tag="std")
                nc.scalar.activation(std[:], mng[:, 1, :], AF.Sqrt, bias=epst[:, 0:1])
                istd = spool.tile([P, GB], F32, name="istd", tag="istd")
                nc.vector.reciprocal(istd[:], std[:])
                nm = spool.tile([P, GB], F32, name="nm", tag="nm")
                nc.vector.scalar_tensor_tensor(nm[:], in0=mng[:, 0, :], scalar=-1.0,
                                               in1=istd[:], op0=OP.mult, op1=OP.mult)
                for t in range(g0, g0 + GB):
                    nc.vector.tensor_scalar(out=dst[:, t * B:(t + 1) * B],
                                            in0=hrbuf[:, t * B:(t + 1) * B],
                                            scalar1=istd[:, t - g0:t - g0 + 1],
                                            scalar2=nm[:, t - g0:t - g0 + 1],
                                            op0=OP.mult, op1=OP.add)

        phase(e1s, lambda t: xs[:, (t // 4) * B:(t // 4 + 1) * B], b1t, h1n, hr)
        phase(w2s, lambda t: h1n[:, t * B:(t + 1) * B], b2t, h2n, hr)

        # ---- layer 3: partial z[b, t] over this core's hidden shard ------
        psZ = [[psZp.tile([P, 512], F32, name=f"psZ{bh}{th}", tag=f"psZ{bh}{th}")
                for th in range(2)] for bh in range(2)]
        for t in range(NT):
            for bh in range(2):
                lhsT = h2n[:, t * B + bh * P: t * B + (bh + 1) * P]
                for th in range(2):
                    nc.tensor.matmul(psZ[bh][th][:], lhsT=lhsT,
                                     rhs=w3s[:, t * N_TF + th * 512: t * N_TF + (th + 1) * 512],
                                     start=(t == 0), stop=(t == NT - 1))

        zpart = apool.tile([P, 2 * N_TF], BF, name="zpart")
        for bh in range(2):
            for th in range(2):
                nc.vector.tensor_copy(
                    zpart[:, bh * N_TF + th * 512: bh * N_TF + (th + 1) * 512],
                    psZ[bh][th][:])

        # transpose to z^T [1024 tf, 256 b], bounce to DRAM
        zinT = dpool.tile([N_TF, B], BF, name="zinT")
        for tt in range(N_TF // P):
            zTs = ztpool.tile([P, B], BF, name="zTs", tag="zTs")
            for bh in range(2):
                pst = psTp.tile([P, P], BF, name="pst", tag="pst")
                nc.tensor.transpose(pst[:], in_=zpart[:, bh * N_TF + tt * P: bh * N_TF + (tt + 1) * P],
                                    identity=idt[:])
                nc.vector.tensor_copy(zTs[:, bh * P:(bh + 1) * P], pst[:])
            nc.gpsimd.dma_start(zinT[tt * P:(tt + 1) * P, :], zTs[:])

        # AllToAll: core c receives, from each rank j, rank j's rows [128c,128c+128)
        za = dpool.tile([N_TF, B], BF, name="za")
        nc.gpsimd.collective_compute(
            "AllToAll", OP.bypass,
            replica_groups=[list(range(NCORES))],
            ins=[zinT.opt()], outs=[za.opt()])

        # load the 8 partial slices as [128, 8*256] and reduce in fp32
        zsl8 = ztpool.tile([P, NCORES * B], BF, name="zsl8", tag="zsl8")
        nc.gpsimd.dma_start(zsl8[:].rearrange("p (j b) -> p j b", j=NCORES),
                            za[:].rearrange("(j p) b -> p j b", p=P))
        zs = ztpool.tile([P, B], F32, name="zs", tag="zs")
        nc.vector.reduce_sum(zs[:], zsl8[:].rearrange("p (j b) -> p b j", j=NCORES),
                             axis=mybir.AxisListType.X)

        # ---- BN3 on the local TF slice -----------------------------------
        st6 = spool.tile([P, 6], F32, name="st6", tag="st6")
        nc.vector.bn_stats(st6[:], zs[:])
        mv3 = spool.tile([P, 2], F32, name="mv3", tag="mv3")
        nc.vector.bn_aggr(mv3[:], st6[:])
        std3 = spool.tile([P, 1], F32, name="std3", tag="std3")
        nc.scalar.activation(std3[:], mv3[:, 1:2], AF.Sqrt, bias=epst[:, 0:1])
        istd3 = spool.tile([P, 1], F32, name="istd3", tag="istd3")
        nc.vector.reciprocal(istd3[:], std3[:])
        nm3 = spool.tile([P, 1], F32, name="nm3", tag="nm3")
        nc.vector.scalar_tensor_tensor(nm3[:], in0=mv3[:, 0:1], scalar=-1.0,
                                       in1=istd3[:], op0=OP.mult, op1=OP.mult)
        ofin = ztpool.tile([P, B], F32, name="ofin", tag="ofin")
        nc.vector.tensor_scalar(out=ofin[:], in0=zs[:], scalar1=istd3[:],
                                scalar2=nm3[:], op0=OP.mult, op1=OP.add)
        nc.sync.dma_start(outT[:], ofin[:])

    nc.compile()
    return nc


def _pack_inputs(features, w1, b1, w2, b2, w3, b3,
                 rows1, cols1, rows2, cols2, rows3, cols3):
    """Host-side packing into per-core contiguous [128, N] tile layouts."""
    f32 = np.float32
    features = np.asarray(features, f32)
    w1 = np.asarray(w1, f32); b1 = np.asarray(b1, f32)
    w2 = np.asarray(w2, f32); b2 = np.asarray(b2, f32)
    w3 = np.asarray(w3, f32)
    rows1 = np.asarray(rows1); cols1 = np.asarray(cols1)
    rows2 = np.asarray(rows2); cols2 = np.asarray(cols2)
    rows3 = np.asarray(rows3); cols3 = np.asarray(cols3)

    w1r = np.empty(HID, f32); w1r[rows1] = w1
    c1r = np.empty(HID, np.int64); c1r[rows1] = cols1

    order2 = np.argsort(rows2, kind="stable")
    r2 = rows2[order2]; c2 = cols2[order2]; v2 = w2[order2]

    W3d = np.zeros((HID, N_TF), f32)
    np.add.at(W3d, (cols3.astype(np.int64), rows3.astype(np.int64)), w3)

    in_maps = []
    for c in range(NCORES):
        hbase = c * HSH
        gbase = c * GSH
        # xd[p, g*B+b] = features[b, gbase + g*128 + p]
        xd = np.ascontiguousarray(
            features[:, gbase:gbase + GSH].T.reshape(NGT, P, B).transpose(1, 0, 2)
            .reshape(P, NGT * B)).astype(BF16)

        e1 = np.zeros((NT, P, P), f32)
        w2t = np.zeros((NT, P, P), f32)
        for t in range(NT):
            R0 = hbase + t * P
            G0 = gbase + (t // 4) * P
            rows = np.arange(R0, R0 + P)
            e1[t][c1r[rows] - G0, np.arange(P)] = w1r[rows]
            es = slice(WM * R0, WM * (R0 + P))
            np.add.at(w2t[t], (c2[es] - R0, r2[es] - R0), v2[es])

        w3t = W3d[hbase:hbase + HSH].reshape(NT, P, N_TF)

        in_maps.append({
            "xd": xd,
            "e1d": np.ascontiguousarray(e1.transpose(1, 0, 2).reshape(P, NT * P)).astype(BF16),
            "w2d": np.ascontiguousarray(w2t.transpose(1, 0, 2).reshape(P, NT * P)).astype(BF16),
            "w3d": np.ascontiguousarray(w3t.transpose(1, 0, 2).reshape(P, NT * N_TF)).astype(BF16),
            "b1d": np.ascontiguousarray(b1[hbase:hbase + HSH].reshape(NT, P).T),
            "b2d": np.ascontiguousarray(b2[hbase:hbase + HSH].reshape(NT, P).T),
        })
    return in_maps


def kernel(**inputs) -> np.ndarray:
    global LAST_RESULT
    if "nc" not in _cache:
        _cache["nc"] = _build_graph()
    nc = _cache["nc"]

    in_maps = _pack_inputs(**inputs)
    # b3 is dropped: BN3 subtracts the per-TF batch mean, so a per-TF constant
    # bias cancels exactly.

    res = bass_utils.run_bass_kernel_spmd(
        nc, in_maps, core_ids=list(range(NCORES)), trace=TRACE)
    LAST_RESULT = res

    outT = np.concatenate([res.results[c]["outT"] for c in range(NCORES)], axis=0)
    return np.ascontiguousarray(outT.T.astype(np.float32))
